# revision 8
# baseline (speedup 1.0000x reference)
"""VQ-VAE forward kernel for Trainium2 (8 NeuronCores, data-parallel over batch).

Reference computation (see problem):
    h      = relu(x @ W1.T + b1)
    latent = h @ W2.T + b2
    codes  = latent.reshape(-1, 64)
    idx    = argmin_k ||codes - codebook[k]||^2
    quant  = codebook[idx]
    vq_loss = 1.25 * mean((quant - codes)^2)
    perplexity = exp(entropy(histogram(idx)))
    recons = relu(quant.reshape(B, F) @ W3.T + b3) @ W4.T + b4
    returns (recons, vq_loss, perplexity)

Strategy:
  - Shard batch (4096) over 8 cores, 512 rows each; weights replicated.
  - All activations kept transposed on-chip: [feature(partition), batch(free)].
  - 4 big matmuls in bf16 (PE 1 cycle/row), N=512.
  - VQ: scores = codes @ cb.T via PE with a block-diagonal zero-padded
    codebook (2 groups of 64 dims share one 128-partition lhsT slice);
    argmax via DVE tensor_tensor_reduce(+max) + max_index; one-hot via
    iota==idx; gather quant = onehot.T @ cb via PE transposes + matmul
    with a block-diagonal codebook table.
  - Scalar stats (vq_loss pieces, histogram) are computed from per-core
    partials (sum of lat^2, sum of best scores, per-code idx) on the host.
"""

import json
import os
import sys
import types

import numpy as np
import ml_dtypes

# This container ships no antenv.axon_hooks (NTFF profiling); stub it so
# run_bass_kernel_spmd(trace=True) degrades gracefully instead of crashing.
if "antenv.axon_hooks" not in sys.modules:
    _m = types.ModuleType("antenv.axon_hooks")
    _m.get_axon_ntff_profile_hook = lambda: None
    sys.modules["antenv.axon_hooks"] = _m

import concourse.bass as bass
import concourse.mybir as mybir
import concourse.tile as tile
from concourse.bass_utils import run_bass_kernel_spmd
from concourse.masks import make_identity

F32 = mybir.dt.float32
BF16 = mybir.dt.bfloat16
U32 = mybir.dt.uint32

N_CORES = 8
B = 4096          # global batch
F = 4096          # feature dim
BS = B // N_CORES  # 512 batch rows per core
KCB = 512         # codebook entries
D = 64            # embedding dim
GROUPS = F // D   # 64 groups per latent row
NJT = F // 128    # 32 feature tiles
NKT = F // 128    # 32 contraction tiles
NIT = BS // 128   # 4 batch tiles per core
COMMIT = 0.25

_NEG_INF = -3.0e38

# ---------------------------------------------------------------------------
# walrus in this env accepts at most ONE sync-wait per instruction; Tile emits
# several. Split extras onto NoOps (same engine, just before the instruction).
_orig_to_json = bass.Bass.to_json_bytes
_split_ctr = [0]


def _split_multiwaits(bir: bytes) -> bytes:
    d = json.loads(bir)
    for fn in d["functions"]:
        for bb in fn["blocks"]:
            out = []
            for inst in bb["instructions"]:
                si = inst.get("sync_info")
                waits = (si or {}).get("on_wait") or []
                if len(waits) > 1:
                    for w in waits[:-1]:
                        _split_ctr[0] += 1
                        out.append({
                            "engine": inst["engine"], "ins": [], "outs": [],
                            "name": f"IWSPLIT-{_split_ctr[0]}", "opcode": "NoOp",
                            "sync_info": {"on_update": [], "on_wait": [w]},
                        })
                    si["on_wait"] = [waits[-1]]
                out.append(inst)
            bb["instructions"] = out
    return json.dumps(d).encode()


def _patched_to_json(self, *a, **k):
    return _split_multiwaits(_orig_to_json(self, *a, **k))


bass.Bass.to_json_bytes = _patched_to_json
# ---------------------------------------------------------------------------


def _emit(nc, tc, io):
    """Emit the per-core program. io: dict of dram APs."""
    ctx_pools = []

    def pool(name, bufs, space="SBUF"):
        p = tc.tile_pool(name=name, bufs=bufs, space=space)
        ctx_pools.append(p)
        return p.__enter__()

    const = pool("const", 1)
    bigx = pool("bigx", 1)      # x, later reused for q (tag-shared)
    bigh = pool("bigh", 1)      # h, later reused for h3
    biglat = pool("biglat", 1)
    wpool = pool("wpool", 3)
    outp = pool("outp", 3)
    scorep = pool("scorep", 3)
    ohp = pool("ohp", 3)
    ohtp = pool("ohtp", 3)
    m8p = pool("m8p", 4)
    i8p = pool("i8p", 4)
    scr = pool("scr", 2)
    acc = pool("acc", 1)
    ps_mm = pool("ps_mm", 2, space="PSUM")
    ps_sc = pool("ps_sc", 2, space="PSUM")
    ps_oh = pool("ps_oh", 2, space="PSUM")
    ps_q = pool("ps_q", 2, space="PSUM")

    # ---- constants ----
    cbr2 = const.tile([128, 2 * KCB], BF16)   # block-diag codebook.T for scores
    nc.sync.dma_start(cbr2[:], io["cbr2"][:])
    cbl2 = const.tile([128, 8 * 128], BF16)   # block-diag codebook for gather
    nc.sync.dma_start(cbl2[:], io["cbl2"][:])
    ncb = const.tile([128, KCB], F32)         # -0.5*||cb_k||^2, replicated rows
    nc.sync.dma_start(ncb[:], io["ncb"][:])
    iota = const.tile([128, KCB], F32)        # 0..511 along free, replicated
    nc.sync.dma_start(iota[:], io["iota"][:])
    bsb = {}
    for l in (1, 2, 3, 4):
        bsb[l] = const.tile([128, NJT], F32, tag=f"b{l}", name=f"b{l}sb")
        nc.sync.dma_start(bsb[l][:], io[f"b{l}"][:])
    ident = const.tile([128, 128], BF16)
    make_identity(nc, ident[:])

    # ---- accumulators / small outputs ----
    idx_sb = acc.tile([128, NIT * GROUPS], F32, tag="idx")
    amax_sb = acc.tile([128, NIT * GROUPS], F32, tag="amax")
    sc2_sb = acc.tile([128, NJT], F32, tag="sc2")

    # ---- x load ----
    x_sb = bigx.tile([128, NKT * BS], BF16, tag="xq")
    for c in range(4):
        w = NKT * BS // 4
        nc.sync.dma_start(x_sb[:, c * w:(c + 1) * w], io["xb"][:, c * w:(c + 1) * w])

    def dense_layer(src_sb, wkey, bias_l, dst_cb):
        """dst_cb(jt, ps) consumes the psum tile for feature-tile jt."""
        for jt in range(NJT):
            w_sb = wpool.tile([128, F], BF16, tag="w")
            for c in range(4):
                nc.sync.dma_start(
                    w_sb[:, c * 1024:(c + 1) * 1024],
                    io[wkey][jt][:, c * 1024:(c + 1) * 1024],
                )
            ps = ps_mm.tile([128, BS], F32, tag="mm")
            for kt in range(NKT):
                nc.tensor.matmul(
                    ps[:],
                    lhsT=w_sb[:, kt * 128:(kt + 1) * 128],
                    rhs=src_sb[:, kt * BS:(kt + 1) * BS],
                    start=(kt == 0),
                    stop=(kt == NKT - 1),
                )
            dst_cb(jt, ps)

    # ---- layer 1: h = relu(x @ W1.T + b1) ----
    h_sb = bigh.tile([128, NJT * BS], BF16, tag="h")

    def l1_out(jt, ps):
        nc.scalar.activation(
            h_sb[:, jt * BS:(jt + 1) * BS], ps[:],
            mybir.ActivationFunctionType.Relu, bias=bsb[1][:, jt:jt + 1],
        )

    dense_layer(x_sb, "w1", 1, l1_out)

    # ---- layer 2 + VQ ----
    lat_sb = biglat.tile([128, NJT * BS], BF16, tag="lat")
    q_sb = bigx.tile([128, NJT * BS], BF16, tag="xq")  # reuses x slot

    def vq_tile(jt, it):
        """VQ for the 128 batch cols `it` of both groups in feature-tile jt."""
        lat2 = lat_sb[:, jt * BS + it * 128: jt * BS + (it + 1) * 128]
        ps_both = []
        for half in range(2):
            ps_s = ps_sc.tile([128, KCB], F32, tag="sc")
            nc.tensor.matmul(
                ps_s[:], lhsT=lat2, rhs=cbr2[:, half * KCB:(half + 1) * KCB],
                start=True, stop=True,
            )
            ps_both.append(ps_s)
        oh_both = []
        for half in range(2):
            g = 2 * jt + half
            col = it * GROUPS + g
            sc_sb = scorep.tile([128, KCB], F32, tag="scs")
            nc.vector.tensor_add(sc_sb[:], ps_both[half][:], ncb[:])
            maxv8 = m8p.tile([128, 8], F32, tag="m8")
            nc.vector.max(maxv8[:], sc_sb[:])
            idx8 = i8p.tile([128, 8], U32, tag="i8")
            nc.vector.max_index(idx8[:], maxv8[:], sc_sb[:])
            nc.vector.tensor_copy(idx_sb[:, col:col + 1], idx8[:, 0:1])
            nc.scalar.copy(amax_sb[:, col:col + 1], maxv8[:, 0:1])
            oh = ohp.tile([128, KCB], BF16, tag="oh")
            nc.vector.tensor_scalar(
                out=oh[:], in0=iota[:], scalar1=idx_sb[:, col:col + 1],
                scalar2=None, op0=mybir.AluOpType.is_equal,
            )
            oh_both.append(oh)
        # transpose both one-hots into [k, n] layout
        ohT = ohtp.tile([128, 8 * 128], BF16, tag="oht")
        for half in range(2):
            ps_t = ps_oh.tile([128, KCB], BF16, tag="oht_ps")
            for s in range(4):
                nc.tensor.transpose(
                    ps_t[:, s * 128:(s + 1) * 128],
                    oh_both[half][:, s * 128:(s + 1) * 128],
                    ident[:],
                )
            nc.scalar.copy(ohT[:, half * KCB:(half + 1) * KCB], ps_t[:])
        # quant.T for both groups: accumulate 8 block-diag codebook matmuls
        ps_g = ps_q.tile([128, 128], F32, tag="q")
        for s in range(8):
            nc.tensor.matmul(
                ps_g[:], lhsT=cbl2[:, s * 128:(s + 1) * 128],
                rhs=ohT[:, s * 128:(s + 1) * 128],
                start=(s == 0), stop=(s == 7),
            )
        nc.scalar.copy(
            q_sb[:, jt * BS + it * 128: jt * BS + (it + 1) * 128], ps_g[:],
        )

    def l2_out(jt, ps):
        nc.scalar.activation(
            lat_sb[:, jt * BS:(jt + 1) * BS], ps[:],
            mybir.ActivationFunctionType.Identity, bias=bsb[2][:, jt:jt + 1],
        )
        scr2 = scr.tile([128, BS], BF16, tag="scr2")
        nc.scalar.activation(
            scr2[:], ps[:], mybir.ActivationFunctionType.Square,
            bias=bsb[2][:, jt:jt + 1], accum_out=sc2_sb[:, jt:jt + 1],
        )
        for it in range(NIT):
            vq_tile(jt, it)

    dense_layer(h_sb, "w2", 2, l2_out)

    # ---- layer 3: h3 = relu(q @ W3.T + b3) ----
    h3_sb = bigh.tile([128, NJT * BS], BF16, tag="h")  # reuses h slot

    def l3_out(jt, ps):
        nc.scalar.activation(
            h3_sb[:, jt * BS:(jt + 1) * BS], ps[:],
            mybir.ActivationFunctionType.Relu, bias=bsb[3][:, jt:jt + 1],
        )

    dense_layer(q_sb, "w3", 3, l3_out)

    # ---- layer 4: recons = h3 @ W4.T + b4 ----
    def l4_out(jt, ps):
        rec = outp.tile([128, BS], F32, tag="rec")
        nc.scalar.activation(
            rec[:], ps[:],
            mybir.ActivationFunctionType.Identity, bias=bsb[4][:, jt:jt + 1],
        )
        nc.sync.dma_start(io["recT"][jt], rec[:])

    dense_layer(h3_sb, "w4", 4, l4_out)

    # ---- small outputs ----
    nc.sync.dma_start(io["idxo"][:], idx_sb[:])
    nc.sync.dma_start(io["amaxo"][:], amax_sb[:])
    nc.sync.dma_start(io["sc2o"][:], sc2_sb[:])

    for p in reversed(ctx_pools):
        p.__exit__(None, None, None)


_cached = {}


def _build():
    if "nc" in _cached:
        return _cached["nc"]
    nc = bass.Bass("TRN2", target_bir_lowering=False, debug=False)
    io = {}
    io["xb"] = nc.dram_tensor("xb", [128, NKT * BS], BF16, kind="ExternalInput").ap()
    for l in (1, 2, 3, 4):
        io[f"w{l}"] = nc.dram_tensor(f"w{l}", [NJT, 128, F], BF16, kind="ExternalInput").ap()
        io[f"b{l}"] = nc.dram_tensor(f"b{l}", [128, NJT], F32, kind="ExternalInput").ap()
    io["cbr2"] = nc.dram_tensor("cbr2", [128, 2 * KCB], BF16, kind="ExternalInput").ap()
    io["cbl2"] = nc.dram_tensor("cbl2", [128, 8 * 128], BF16, kind="ExternalInput").ap()
    io["ncb"] = nc.dram_tensor("ncb", [128, KCB], F32, kind="ExternalInput").ap()
    io["iota"] = nc.dram_tensor("iota", [128, KCB], F32, kind="ExternalInput").ap()
    io["recT"] = nc.dram_tensor("recT", [NJT, 128, BS], F32, kind="ExternalOutput").ap()
    io["idxo"] = nc.dram_tensor("idxo", [128, NIT * GROUPS], F32, kind="ExternalOutput").ap()
    io["amaxo"] = nc.dram_tensor("amaxo", [128, NIT * GROUPS], F32, kind="ExternalOutput").ap()
    io["sc2o"] = nc.dram_tensor("sc2o", [128, NJT], F32, kind="ExternalOutput").ap()
    with tile.TileContext(nc) as tc:
        _emit(nc, tc, io)
    _cached["nc"] = nc
    return nc


def _prep_shared(W1, b1, W2, b2, W3, b3, W4, b4, codebook):
    bf = ml_dtypes.bfloat16
    out = {}
    for l, (W, b) in enumerate(((W1, b1), (W2, b2), (W3, b3), (W4, b4)), start=1):
        W = np.asarray(W, np.float32)
        wb = np.ascontiguousarray(
            W.T.reshape(NKT, 128, NJT, 128).transpose(2, 1, 0, 3).reshape(NJT, 128, F)
        ).astype(bf)
        out[f"w{l}"] = wb
        out[f"b{l}"] = np.ascontiguousarray(
            np.asarray(b, np.float32).reshape(NJT, 128).T
        )
    cb = np.asarray(codebook, np.float32)  # [512, 64]
    cbr2 = np.zeros((128, 2 * KCB), np.float32)
    cbr2[:D, :KCB] = cb.T
    cbr2[D:2 * D, KCB:] = cb.T
    out["cbr2"] = cbr2.astype(bf)
    cbl2 = np.zeros((128, 8 * 128), np.float32)
    for s in range(8):
        half, kb = s // 4, (s % 4) * 128
        blk = cb[kb:kb + 128, :]                     # [128, 64]
        cbl2[:, s * 128 + half * D: s * 128 + half * D + D] = blk
    out["cbl2"] = cbl2.astype(bf)
    cbnorm_half = 0.5 * np.sum(cb.astype(np.float64) ** 2, axis=1).astype(np.float32)
    out["ncb"] = np.ascontiguousarray(np.tile(-cbnorm_half, (128, 1)))
    out["iota"] = np.ascontiguousarray(
        np.tile(np.arange(KCB, dtype=np.float32), (128, 1))
    )
    return out, cbnorm_half


def kernel(x, W1, b1, W2, b2, W3, b3, W4, b4, codebook):
    x = np.asarray(x, np.float32)
    nc = _build()
    shared, cbnorm_half = _prep_shared(W1, b1, W2, b2, W3, b3, W4, b4, codebook)
    bf = ml_dtypes.bfloat16
    in_maps = []
    for c in range(N_CORES):
        xs = x[c * BS:(c + 1) * BS]                   # [512, 4096]
        xb = np.ascontiguousarray(
            xs.T.reshape(NKT, 128, BS).transpose(1, 0, 2).reshape(128, NKT * BS)
        ).astype(bf)
        m = dict(shared)
        m["xb"] = xb
        in_maps.append(m)

    res = run_bass_kernel_spmd(nc, in_maps, list(range(N_CORES)), trace=False)
    kernel._last = res

    recons = np.empty((B, F), np.float32)
    idx_all = np.empty((B // 128, 128, GROUPS), np.int64)
    amax_total = 0.0
    sumc2_total = 0.0
    for c in range(N_CORES):
        r = res.results[c]
        recons[c * BS:(c + 1) * BS] = r["recT"].reshape(F, BS).T
        idx_c = r["idxo"].reshape(128, NIT, GROUPS)
        idx_all[c * NIT:(c + 1) * NIT] = np.transpose(idx_c, (1, 0, 2)).astype(np.int64)
        amax_total += np.sum(r["amaxo"].astype(np.float64))
        sumc2_total += np.sum(r["sc2o"].astype(np.float64))

    idx_flat = idx_all.reshape(-1)
    n_codes = idx_flat.shape[0]
    counts = np.bincount(idx_flat, minlength=KCB).astype(np.float64)

    cb64 = np.asarray(codebook, np.float64)
    cbnorm2 = np.sum(cb64 ** 2, axis=1)
    sum_cq = amax_total + float(counts @ cbnorm_half.astype(np.float64))
    sum_q2 = float(counts @ cbnorm2)
    sse = sumc2_total - 2.0 * sum_cq + sum_q2
    mse = sse / (n_codes * D)
    vq_loss = np.float32((1.0 + COMMIT) * mse)

    avg = counts / n_codes
    perplexity = np.float32(np.exp(-np.sum(avg * np.log(avg + 1e-10))))

    return recons, vq_loss, perplexity


# ---------------------------------------------------------------------------
# Timing helper (dev only): compile once, keep inputs on device, time repeats.
def _make_runner(nc, in_maps):
    import jax
    import concourse.mybir as mb
    from jax.sharding import Mesh, PartitionSpec
    from jax.experimental.shard_map import shard_map
    from concourse.bass2jax import _bass_exec_p, install_neuronx_cc_hook, partition_id_tensor

    install_neuronx_cc_hook()
    n_cores = len(in_maps)
    in_names, out_names, out_avals, zero_outs = [], [], [], []
    partition_name = nc.partition_id_tensor.name if nc.partition_id_tensor else None
    for alloc in nc.m.functions[0].allocations:
        if not isinstance(alloc, mb.MemoryLocationSet):
            continue
        name = alloc.memorylocations[0].name
        if alloc.kind == "ExternalInput":
            if name != partition_name:
                in_names.append(name)
        elif alloc.kind == "ExternalOutput":
            out_names.append(name)
            shape = tuple(alloc.tensor_shape)
            dtype = mb.dt.np(alloc.dtype)
            out_avals.append(jax.core.ShapedArray(shape, dtype))
            zero_outs.append(np.zeros(shape, dtype))
    n_params = len(in_names)
    all_in = list(in_names) + list(out_names)
    if partition_name is not None:
        all_in.append(partition_name)

    def _body(*args):
        operands = list(args)
        if partition_name is not None:
            operands.append(partition_id_tensor())
        outs = _bass_exec_p.bind(
            *operands, out_avals=tuple(out_avals), in_names=tuple(all_in),
            out_names=tuple(out_names), lowering_input_output_aliases=(),
            sim_require_finite=True, sim_require_nnan=True, nc=nc,
        )
        return tuple(outs)

    devices = jax.devices()[:n_cores]
    mesh = Mesh(np.asarray(devices), ("core",))
    n_ops = n_params + len(out_names)
    sharded = jax.jit(
        shard_map(_body, mesh=mesh, in_specs=(PartitionSpec("core"),) * n_ops,
                  out_specs=(PartitionSpec("core"),) * len(out_names), check_rep=False),
        keep_unused=True,
    )
    concat_in = [
        np.concatenate([np.asarray(in_maps[c][nm]) for c in range(n_cores)], axis=0)
        for nm in in_names
    ]
    concat_zeros = [np.zeros((n_cores * z.shape[0], *z.shape[1:]), z.dtype) for z in zero_outs]
    dev_args = [jax.device_put(a) for a in concat_in + concat_zeros]
    return sharded, dev_args


def benchmark(iters=5, **inputs):
    """Returns list of per-iteration wall times (s) for the main kernel."""
    import time as _time
    import jax
    x = np.asarray(inputs["x"], np.float32)
    nc = _build()
    shared, _ = _prep_shared(
        inputs["W1"], inputs["b1"], inputs["W2"], inputs["b2"],
        inputs["W3"], inputs["b3"], inputs["W4"], inputs["b4"], inputs["codebook"],
    )
    bf = ml_dtypes.bfloat16
    in_maps = []
    for c in range(N_CORES):
        xs = x[c * BS:(c + 1) * BS]
        xb = np.ascontiguousarray(
            xs.T.reshape(NKT, 128, BS).transpose(1, 0, 2).reshape(128, NKT * BS)
        ).astype(bf)
        m = dict(shared)
        m["xb"] = xb
        in_maps.append(m)
    fn, dev_args = _make_runner(nc, in_maps)
    times = []
    for _ in range(iters + 1):
        t0 = _time.perf_counter()
        r = fn(*dev_args)
        jax.block_until_ready(r)
        times.append(_time.perf_counter() - t0)
    return times[1:]  # drop compile/warmup


def benchmark_overhead(iters=5):
    """Times a trivial 8-core kernel through the same path (dispatch floor)."""
    import time as _time
    import jax
    import concourse.tile as _tile
    nc = bass.Bass("TRN2", target_bir_lowering=False, debug=False)
    a = nc.dram_tensor("a", [128, 512], F32, kind="ExternalInput").ap()
    y = nc.dram_tensor("y", [128, 512], F32, kind="ExternalOutput").ap()
    with _tile.TileContext(nc) as tc:
        with tc.tile_pool(name="p", bufs=2) as pool:
            t = pool.tile([128, 512], F32)
            nc.sync.dma_start(t[:], a[:])
            nc.sync.dma_start(y[:], t[:])
    in_maps = [{"a": np.zeros((128, 512), np.float32)} for _ in range(N_CORES)]
    fn, dev_args = _make_runner(nc, in_maps)
    times = []
    for _ in range(iters + 1):
        t0 = _time.perf_counter()
        r = fn(*dev_args)
        jax.block_until_ready(r)
        times.append(_time.perf_counter() - t0)
    return times[1:]


# revision 9
# speedup vs baseline: 19.1559x; 19.1559x over previous
"""VQ-VAE forward kernel for Trainium2 (8 NeuronCores, data-parallel over batch).

Reference computation (see problem):
    h      = relu(x @ W1.T + b1)
    latent = h @ W2.T + b2
    codes  = latent.reshape(-1, 64)
    idx    = argmin_k ||codes - codebook[k]||^2
    quant  = codebook[idx]
    vq_loss = 1.25 * mean((quant - codes)^2)
    perplexity = exp(entropy(histogram(idx)))
    recons = relu(quant.reshape(B, F) @ W3.T + b3) @ W4.T + b4
    returns (recons, vq_loss, perplexity)

Strategy:
  - Shard batch (4096) over 8 cores, 512 rows each; weights replicated.
  - All activations kept transposed on-chip: [feature(partition), batch(free)].
  - 4 big matmuls in bf16 (PE 1 cycle/row), N=512.
  - VQ: scores = codes @ cb.T via PE with a block-diagonal zero-padded
    codebook (2 groups of 64 dims share one 128-partition lhsT slice);
    argmax via DVE tensor_tensor_reduce(+max) + max_index; one-hot via
    iota==idx; gather quant = onehot.T @ cb via PE transposes + matmul
    with a block-diagonal codebook table.
  - Scalar stats (vq_loss pieces, histogram) are computed from per-core
    partials (sum of lat^2, sum of best scores, per-code idx) on the host.
"""

import json
import os
import sys
import types

import numpy as np
import ml_dtypes

# This container ships no antenv.axon_hooks (NTFF profiling); stub it so
# run_bass_kernel_spmd(trace=True) degrades gracefully instead of crashing.
if "antenv.axon_hooks" not in sys.modules:
    _m = types.ModuleType("antenv.axon_hooks")
    _m.get_axon_ntff_profile_hook = lambda: None
    sys.modules["antenv.axon_hooks"] = _m

import concourse.bass as bass
import concourse.mybir as mybir
import concourse.tile as tile
from concourse.bass_utils import run_bass_kernel_spmd
from concourse.masks import make_identity

F32 = mybir.dt.float32
BF16 = mybir.dt.bfloat16
U32 = mybir.dt.uint32

N_CORES = 8
B = 4096          # global batch
F = 4096          # feature dim
BS = B // N_CORES  # 512 batch rows per core
KCB = 512         # codebook entries
D = 64            # embedding dim
GROUPS = F // D   # 64 groups per latent row
NJT = F // 128    # 32 feature tiles
NKT = F // 128    # 32 contraction tiles
NIT = BS // 128   # 4 batch tiles per core
COMMIT = 0.25

_NEG_INF = -3.0e38

# ---------------------------------------------------------------------------
# walrus in this env accepts at most ONE sync-wait per instruction; Tile emits
# several. Split extras onto NoOps (same engine, just before the instruction).
_orig_to_json = bass.Bass.to_json_bytes
_split_ctr = [0]


def _split_multiwaits(bir: bytes) -> bytes:
    d = json.loads(bir)
    for fn in d["functions"]:
        for bb in fn["blocks"]:
            out = []
            for inst in bb["instructions"]:
                si = inst.get("sync_info")
                waits = (si or {}).get("on_wait") or []
                if len(waits) > 1:
                    for w in waits[:-1]:
                        _split_ctr[0] += 1
                        out.append({
                            "engine": inst["engine"], "ins": [], "outs": [],
                            "name": f"IWSPLIT-{_split_ctr[0]}", "opcode": "NoOp",
                            "sync_info": {"on_update": [], "on_wait": [w]},
                        })
                    si["on_wait"] = [waits[-1]]
                out.append(inst)
            bb["instructions"] = out
    return json.dumps(d).encode()


def _patched_to_json(self, *a, **k):
    return _split_multiwaits(_orig_to_json(self, *a, **k))


bass.Bass.to_json_bytes = _patched_to_json
# ---------------------------------------------------------------------------


def _emit(nc, tc, io):
    """Emit the per-core program. io: dict of dram APs."""
    ctx_pools = []

    def pool(name, bufs, space="SBUF"):
        p = tc.tile_pool(name=name, bufs=bufs, space=space)
        ctx_pools.append(p)
        return p.__enter__()

    const = pool("const", 1)
    bigx = pool("bigx", 1)      # x, later reused for q (tag-shared)
    bigh = pool("bigh", 1)      # h, later reused for h3
    biglat = pool("biglat", 1)
    wpool = pool("wpool", 3)
    outp = pool("outp", 3)
    scorep = pool("scorep", 3)
    ohp = pool("ohp", 3)
    ohtp = pool("ohtp", 3)
    m8p = pool("m8p", 4)
    i8p = pool("i8p", 4)
    scr = pool("scr", 2)
    acc = pool("acc", 1)
    ps_mm = pool("ps_mm", 2, space="PSUM")
    ps_sc = pool("ps_sc", 2, space="PSUM")
    ps_oh = pool("ps_oh", 2, space="PSUM")
    ps_q = pool("ps_q", 2, space="PSUM")

    # ---- constants ----
    cbr2 = const.tile([128, 2 * KCB], BF16)   # block-diag codebook.T for scores
    nc.sync.dma_start(cbr2[:], io["cbr2"][:])
    cbl2 = const.tile([128, 8 * 128], BF16)   # block-diag codebook for gather
    nc.sync.dma_start(cbl2[:], io["cbl2"][:])
    ncb = const.tile([128, KCB], F32)         # -0.5*||cb_k||^2, replicated rows
    nc.sync.dma_start(ncb[:], io["ncb"][:])
    iota = const.tile([128, KCB], F32)        # 0..511 along free, replicated
    nc.sync.dma_start(iota[:], io["iota"][:])
    bsb = {}
    for l in (1, 2, 3, 4):
        bsb[l] = const.tile([128, NJT], F32, tag=f"b{l}", name=f"b{l}sb")
        nc.sync.dma_start(bsb[l][:], io[f"b{l}"][:])
    ident = const.tile([128, 128], BF16)
    make_identity(nc, ident[:])

    # ---- accumulators / small outputs ----
    idx_sb = acc.tile([128, NIT * GROUPS], F32, tag="idx")
    amax_sb = acc.tile([128, NIT * GROUPS], F32, tag="amax")
    sc2_sb = acc.tile([128, NJT], F32, tag="sc2")

    # ---- x load ----
    x_sb = bigx.tile([128, NKT * BS], BF16, tag="xq")
    for c in range(4):
        w = NKT * BS // 4
        nc.sync.dma_start(x_sb[:, c * w:(c + 1) * w], io["xb"][:, c * w:(c + 1) * w])

    def dense_layer(src_sb, wkey, bias_l, dst_cb):
        """dst_cb(jt, ps) consumes the psum tile for feature-tile jt."""
        for jt in range(NJT):
            w_sb = wpool.tile([128, F], BF16, tag="w")
            for c in range(4):
                nc.sync.dma_start(
                    w_sb[:, c * 1024:(c + 1) * 1024],
                    io[wkey][jt][:, c * 1024:(c + 1) * 1024],
                )
            ps = ps_mm.tile([128, BS], F32, tag="mm")
            for kt in range(NKT):
                nc.tensor.matmul(
                    ps[:],
                    lhsT=w_sb[:, kt * 128:(kt + 1) * 128],
                    rhs=src_sb[:, kt * BS:(kt + 1) * BS],
                    start=(kt == 0),
                    stop=(kt == NKT - 1),
                )
            dst_cb(jt, ps)

    # ---- layer 1: h = relu(x @ W1.T + b1) ----
    h_sb = bigh.tile([128, NJT * BS], BF16, tag="h")

    def l1_out(jt, ps):
        nc.scalar.activation(
            h_sb[:, jt * BS:(jt + 1) * BS], ps[:],
            mybir.ActivationFunctionType.Relu, bias=bsb[1][:, jt:jt + 1],
        )

    dense_layer(x_sb, "w1", 1, l1_out)

    # ---- layer 2 + VQ ----
    lat_sb = biglat.tile([128, NJT * BS], BF16, tag="lat")
    q_sb = bigx.tile([128, NJT * BS], BF16, tag="xq")  # reuses x slot

    def vq_tile(jt, it):
        """VQ for the 128 batch cols `it` of both groups in feature-tile jt."""
        lat2 = lat_sb[:, jt * BS + it * 128: jt * BS + (it + 1) * 128]
        ps_both = []
        for half in range(2):
            ps_s = ps_sc.tile([128, KCB], F32, tag="sc")
            nc.tensor.matmul(
                ps_s[:], lhsT=lat2, rhs=cbr2[:, half * KCB:(half + 1) * KCB],
                start=True, stop=True,
            )
            ps_both.append(ps_s)
        oh_both = []
        for half in range(2):
            g = 2 * jt + half
            col = it * GROUPS + g
            sc_sb = scorep.tile([128, KCB], F32, tag="scs")
            nc.vector.tensor_add(sc_sb[:], ps_both[half][:], ncb[:])
            maxv8 = m8p.tile([128, 8], F32, tag="m8")
            nc.vector.max(maxv8[:], sc_sb[:])
            idx8 = i8p.tile([128, 8], U32, tag="i8")
            nc.vector.max_index(idx8[:], maxv8[:], sc_sb[:])
            nc.vector.tensor_copy(idx_sb[:, col:col + 1], idx8[:, 0:1])
            nc.scalar.copy(amax_sb[:, col:col + 1], maxv8[:, 0:1])
            oh = ohp.tile([128, KCB], BF16, tag="oh")
            nc.vector.tensor_scalar(
                out=oh[:], in0=iota[:], scalar1=idx_sb[:, col:col + 1],
                scalar2=None, op0=mybir.AluOpType.is_equal,
            )
            oh_both.append(oh)
        # transpose both one-hots into [k, n] layout
        ohT = ohtp.tile([128, 8 * 128], BF16, tag="oht")
        for half in range(2):
            ps_t = ps_oh.tile([128, KCB], BF16, tag="oht_ps")
            for s in range(4):
                nc.tensor.transpose(
                    ps_t[:, s * 128:(s + 1) * 128],
                    oh_both[half][:, s * 128:(s + 1) * 128],
                    ident[:],
                )
            nc.scalar.copy(ohT[:, half * KCB:(half + 1) * KCB], ps_t[:])
        # quant.T for both groups: accumulate 8 block-diag codebook matmuls
        ps_g = ps_q.tile([128, 128], F32, tag="q")
        for s in range(8):
            nc.tensor.matmul(
                ps_g[:], lhsT=cbl2[:, s * 128:(s + 1) * 128],
                rhs=ohT[:, s * 128:(s + 1) * 128],
                start=(s == 0), stop=(s == 7),
            )
        nc.scalar.copy(
            q_sb[:, jt * BS + it * 128: jt * BS + (it + 1) * 128], ps_g[:],
        )

    def l2_out(jt, ps):
        nc.scalar.activation(
            lat_sb[:, jt * BS:(jt + 1) * BS], ps[:],
            mybir.ActivationFunctionType.Identity, bias=bsb[2][:, jt:jt + 1],
        )
        scr2 = scr.tile([128, BS], BF16, tag="scr2")
        nc.scalar.activation(
            scr2[:], ps[:], mybir.ActivationFunctionType.Square,
            bias=bsb[2][:, jt:jt + 1], accum_out=sc2_sb[:, jt:jt + 1],
        )
        for it in range(NIT):
            vq_tile(jt, it)

    dense_layer(h_sb, "w2", 2, l2_out)

    # ---- layer 3: h3 = relu(q @ W3.T + b3) ----
    h3_sb = bigh.tile([128, NJT * BS], BF16, tag="h")  # reuses h slot

    def l3_out(jt, ps):
        nc.scalar.activation(
            h3_sb[:, jt * BS:(jt + 1) * BS], ps[:],
            mybir.ActivationFunctionType.Relu, bias=bsb[3][:, jt:jt + 1],
        )

    dense_layer(q_sb, "w3", 3, l3_out)

    # ---- layer 4: recons = h3 @ W4.T + b4 ----
    def l4_out(jt, ps):
        rec = outp.tile([128, BS], F32, tag="rec")
        nc.scalar.activation(
            rec[:], ps[:],
            mybir.ActivationFunctionType.Identity, bias=bsb[4][:, jt:jt + 1],
        )
        nc.sync.dma_start(io["recT"][jt], rec[:])

    dense_layer(h3_sb, "w4", 4, l4_out)

    # ---- small outputs ----
    nc.sync.dma_start(io["idxo"][:], idx_sb[:])
    nc.sync.dma_start(io["amaxo"][:], amax_sb[:])
    nc.sync.dma_start(io["sc2o"][:], sc2_sb[:])

    for p in reversed(ctx_pools):
        p.__exit__(None, None, None)


_cached = {}


def _build():
    if "nc" in _cached:
        return _cached["nc"]
    nc = bass.Bass("TRN2", target_bir_lowering=False, debug=False)
    io = {}
    io["xb"] = nc.dram_tensor("xb", [128, NKT * BS], BF16, kind="ExternalInput").ap()
    for l in (1, 2, 3, 4):
        io[f"w{l}"] = nc.dram_tensor(f"w{l}", [NJT, 128, F], BF16, kind="ExternalInput").ap()
        io[f"b{l}"] = nc.dram_tensor(f"b{l}", [128, NJT], F32, kind="ExternalInput").ap()
    io["cbr2"] = nc.dram_tensor("cbr2", [128, 2 * KCB], BF16, kind="ExternalInput").ap()
    io["cbl2"] = nc.dram_tensor("cbl2", [128, 8 * 128], BF16, kind="ExternalInput").ap()
    io["ncb"] = nc.dram_tensor("ncb", [128, KCB], F32, kind="ExternalInput").ap()
    io["iota"] = nc.dram_tensor("iota", [128, KCB], F32, kind="ExternalInput").ap()
    io["recT"] = nc.dram_tensor("recT", [NJT, 128, BS], F32, kind="ExternalOutput").ap()
    io["idxo"] = nc.dram_tensor("idxo", [128, NIT * GROUPS], F32, kind="ExternalOutput").ap()
    io["amaxo"] = nc.dram_tensor("amaxo", [128, NIT * GROUPS], F32, kind="ExternalOutput").ap()
    io["sc2o"] = nc.dram_tensor("sc2o", [128, NJT], F32, kind="ExternalOutput").ap()
    with tile.TileContext(nc) as tc:
        _emit(nc, tc, io)
    _cached["nc"] = nc
    return nc


def _prep_shared(W1, b1, W2, b2, W3, b3, W4, b4, codebook):
    bf = ml_dtypes.bfloat16
    out = {}
    for l, (W, b) in enumerate(((W1, b1), (W2, b2), (W3, b3), (W4, b4)), start=1):
        W = np.asarray(W, np.float32)
        wb = np.ascontiguousarray(
            W.T.reshape(NKT, 128, NJT, 128).transpose(2, 1, 0, 3).reshape(NJT, 128, F)
        ).astype(bf)
        out[f"w{l}"] = wb
        out[f"b{l}"] = np.ascontiguousarray(
            np.asarray(b, np.float32).reshape(NJT, 128).T
        )
    cb = np.asarray(codebook, np.float32)  # [512, 64]
    cbr2 = np.zeros((128, 2 * KCB), np.float32)
    cbr2[:D, :KCB] = cb.T
    cbr2[D:2 * D, KCB:] = cb.T
    out["cbr2"] = cbr2.astype(bf)
    cbl2 = np.zeros((128, 8 * 128), np.float32)
    for s in range(8):
        half, kb = s // 4, (s % 4) * 128
        blk = cb[kb:kb + 128, :]                     # [128, 64]
        cbl2[:, s * 128 + half * D: s * 128 + half * D + D] = blk
    out["cbl2"] = cbl2.astype(bf)
    cbnorm_half = 0.5 * np.sum(cb.astype(np.float64) ** 2, axis=1).astype(np.float32)
    out["ncb"] = np.ascontiguousarray(np.tile(-cbnorm_half, (128, 1)))
    out["iota"] = np.ascontiguousarray(
        np.tile(np.arange(KCB, dtype=np.float32), (128, 1))
    )
    return out, cbnorm_half


def kernel(x, W1, b1, W2, b2, W3, b3, W4, b4, codebook):
    x = np.asarray(x, np.float32)
    nc = _build()
    shared, cbnorm_half = _prep_shared(W1, b1, W2, b2, W3, b3, W4, b4, codebook)
    bf = ml_dtypes.bfloat16
    in_maps = []
    for c in range(N_CORES):
        xs = x[c * BS:(c + 1) * BS]                   # [512, 4096]
        xb = np.ascontiguousarray(
            xs.T.reshape(NKT, 128, BS).transpose(1, 0, 2).reshape(128, NKT * BS)
        ).astype(bf)
        m = dict(shared)
        m["xb"] = xb
        in_maps.append(m)

    res = run_bass_kernel_spmd(nc, in_maps, list(range(N_CORES)), trace=False)
    kernel._last = res

    recons = np.empty((B, F), np.float32)
    idx_all = np.empty((B // 128, 128, GROUPS), np.int64)
    amax_total = 0.0
    sumc2_total = 0.0
    for c in range(N_CORES):
        r = res.results[c]
        recons[c * BS:(c + 1) * BS] = r["recT"].reshape(F, BS).T
        idx_c = r["idxo"].reshape(128, NIT, GROUPS)
        idx_all[c * NIT:(c + 1) * NIT] = np.transpose(idx_c, (1, 0, 2)).astype(np.int64)
        amax_total += np.sum(r["amaxo"].astype(np.float64))
        sumc2_total += np.sum(r["sc2o"].astype(np.float64))

    idx_flat = idx_all.reshape(-1)
    n_codes = idx_flat.shape[0]
    counts = np.bincount(idx_flat, minlength=KCB).astype(np.float64)

    cb64 = np.asarray(codebook, np.float64)
    cbnorm2 = np.sum(cb64 ** 2, axis=1)
    sum_cq = amax_total + float(counts @ cbnorm_half.astype(np.float64))
    sum_q2 = float(counts @ cbnorm2)
    sse = sumc2_total - 2.0 * sum_cq + sum_q2
    mse = sse / (n_codes * D)
    vq_loss = np.float32((1.0 + COMMIT) * mse)

    avg = counts / n_codes
    perplexity = np.float32(np.exp(-np.sum(avg * np.log(avg + 1e-10))))

    return recons, vq_loss, perplexity


# ---------------------------------------------------------------------------
# Timing helper (dev only): compile once, keep inputs on device, time repeats.
def _make_runner(nc, in_maps):
    import jax
    import concourse.mybir as mb
    from jax.sharding import Mesh, PartitionSpec
    from jax.experimental.shard_map import shard_map
    from concourse.bass2jax import _bass_exec_p, install_neuronx_cc_hook, partition_id_tensor

    install_neuronx_cc_hook()
    n_cores = len(in_maps)
    in_names, out_names, out_avals, zero_outs = [], [], [], []
    partition_name = nc.partition_id_tensor.name if nc.partition_id_tensor else None
    for alloc in nc.m.functions[0].allocations:
        if not isinstance(alloc, mb.MemoryLocationSet):
            continue
        name = alloc.memorylocations[0].name
        if alloc.kind == "ExternalInput":
            if name != partition_name:
                in_names.append(name)
        elif alloc.kind == "ExternalOutput":
            out_names.append(name)
            shape = tuple(alloc.tensor_shape)
            dtype = mb.dt.np(alloc.dtype)
            out_avals.append(jax.core.ShapedArray(shape, dtype))
            zero_outs.append(np.zeros(shape, dtype))
    n_params = len(in_names)
    all_in = list(in_names) + list(out_names)
    if partition_name is not None:
        all_in.append(partition_name)

    def _body(*args):
        operands = list(args)
        if partition_name is not None:
            operands.append(partition_id_tensor())
        outs = _bass_exec_p.bind(
            *operands, out_avals=tuple(out_avals), in_names=tuple(all_in),
            out_names=tuple(out_names), lowering_input_output_aliases=(),
            sim_require_finite=True, sim_require_nnan=True, nc=nc,
        )
        return tuple(outs)

    devices = jax.devices()[:n_cores]
    mesh = Mesh(np.asarray(devices), ("core",))
    n_ops = n_params + len(out_names)
    sharded = jax.jit(
        shard_map(_body, mesh=mesh, in_specs=(PartitionSpec("core"),) * n_ops,
                  out_specs=(PartitionSpec("core"),) * len(out_names), check_rep=False),
        keep_unused=True,
    )
    concat_in = [
        np.concatenate([np.asarray(in_maps[c][nm]) for c in range(n_cores)], axis=0)
        for nm in in_names
    ]
    concat_zeros = [np.zeros((n_cores * z.shape[0], *z.shape[1:]), z.dtype) for z in zero_outs]
    dev_args = [jax.device_put(a) for a in concat_in + concat_zeros]
    return sharded, dev_args


def benchmark(iters=5, **inputs):
    """Returns list of per-iteration wall times (s) for the main kernel."""
    import time as _time
    import jax
    x = np.asarray(inputs["x"], np.float32)
    nc = _build()
    shared, _ = _prep_shared(
        inputs["W1"], inputs["b1"], inputs["W2"], inputs["b2"],
        inputs["W3"], inputs["b3"], inputs["W4"], inputs["b4"], inputs["codebook"],
    )
    bf = ml_dtypes.bfloat16
    in_maps = []
    for c in range(N_CORES):
        xs = x[c * BS:(c + 1) * BS]
        xb = np.ascontiguousarray(
            xs.T.reshape(NKT, 128, BS).transpose(1, 0, 2).reshape(128, NKT * BS)
        ).astype(bf)
        m = dict(shared)
        m["xb"] = xb
        in_maps.append(m)
    fn, dev_args = _make_runner(nc, in_maps)
    times = []
    for _ in range(iters + 1):
        t0 = _time.perf_counter()
        r = fn(*dev_args)
        jax.block_until_ready(r)
        times.append(_time.perf_counter() - t0)
    return times[1:]  # drop compile/warmup


def benchmark_null(iters=5, **inputs):
    """Same input tensors as the real kernel, but near-zero compute.
    Measures the per-call transfer/dispatch cost to subtract."""
    import time as _time
    import jax
    import concourse.tile as _tile
    x = np.asarray(inputs["x"], np.float32)
    shared, _ = _prep_shared(
        inputs["W1"], inputs["b1"], inputs["W2"], inputs["b2"],
        inputs["W3"], inputs["b3"], inputs["W4"], inputs["b4"], inputs["codebook"],
    )
    bf = ml_dtypes.bfloat16
    in_maps = []
    for c in range(N_CORES):
        xs = x[c * BS:(c + 1) * BS]
        xb = np.ascontiguousarray(
            xs.T.reshape(NKT, 128, BS).transpose(1, 0, 2).reshape(128, NKT * BS)
        ).astype(bf)
        m = dict(shared)
        m["xb"] = xb
        in_maps.append(m)

    nc = bass.Bass("TRN2", target_bir_lowering=False, debug=False)
    io = {}
    io["xb"] = nc.dram_tensor("xb", [128, NKT * BS], BF16, kind="ExternalInput").ap()
    for l in (1, 2, 3, 4):
        io[f"w{l}"] = nc.dram_tensor(f"w{l}", [NJT, 128, F], BF16, kind="ExternalInput").ap()
        io[f"b{l}"] = nc.dram_tensor(f"b{l}", [128, NJT], F32, kind="ExternalInput").ap()
    io["cbr2"] = nc.dram_tensor("cbr2", [128, 2 * KCB], BF16, kind="ExternalInput").ap()
    io["cbl2"] = nc.dram_tensor("cbl2", [128, 8 * 128], BF16, kind="ExternalInput").ap()
    io["ncb"] = nc.dram_tensor("ncb", [128, KCB], F32, kind="ExternalInput").ap()
    io["iota"] = nc.dram_tensor("iota", [128, KCB], F32, kind="ExternalInput").ap()
    io["recT"] = nc.dram_tensor("recT", [NJT, 128, BS], F32, kind="ExternalOutput").ap()
    io["idxo"] = nc.dram_tensor("idxo", [128, NIT * GROUPS], F32, kind="ExternalOutput").ap()
    io["amaxo"] = nc.dram_tensor("amaxo", [128, NIT * GROUPS], F32, kind="ExternalOutput").ap()
    io["sc2o"] = nc.dram_tensor("sc2o", [128, NJT], F32, kind="ExternalOutput").ap()
    with _tile.TileContext(nc) as tc:
        with tc.tile_pool(name="p", bufs=2) as pool:
            t = pool.tile([128, NIT * GROUPS], F32)
            nc.sync.dma_start(t[:], io["ncb"][:, :NIT * GROUPS])
            nc.sync.dma_start(io["idxo"][:], t[:])
            nc.sync.dma_start(io["amaxo"][:], t[:])
            t2 = pool.tile([128, NJT], F32)
            nc.sync.dma_start(t2[:], io["ncb"][:, :NJT])
            nc.sync.dma_start(io["sc2o"][:], t2[:])
            for jt in range(NJT):
                t3 = pool.tile([128, BS], F32)
                nc.sync.dma_start(t3[:], io["ncb"][:, :BS])
                nc.sync.dma_start(io["recT"][jt], t3[:])
    fn, dev_args = _make_runner(nc, in_maps)
    times = []
    for _ in range(iters + 1):
        t0 = _time.perf_counter()
        r = fn(*dev_args)
        jax.block_until_ready(r)
        times.append(_time.perf_counter() - t0)
    return times[1:]


def benchmark_overhead(iters=5):
    """Times a trivial 8-core kernel through the same path (dispatch floor)."""
    import time as _time
    import jax
    import concourse.tile as _tile
    nc = bass.Bass("TRN2", target_bir_lowering=False, debug=False)
    a = nc.dram_tensor("a", [128, 512], F32, kind="ExternalInput").ap()
    y = nc.dram_tensor("y", [128, 512], F32, kind="ExternalOutput").ap()
    with _tile.TileContext(nc) as tc:
        with tc.tile_pool(name="p", bufs=2) as pool:
            t = pool.tile([128, 512], F32)
            nc.sync.dma_start(t[:], a[:])
            nc.sync.dma_start(y[:], t[:])
    in_maps = [{"a": np.zeros((128, 512), np.float32)} for _ in range(N_CORES)]
    fn, dev_args = _make_runner(nc, in_maps)
    times = []
    for _ in range(iters + 1):
        t0 = _time.perf_counter()
        r = fn(*dev_args)
        jax.block_until_ready(r)
        times.append(_time.perf_counter() - t0)
    return times[1:]
